# revision 6
# baseline (speedup 1.0000x reference)
"""Trainium2 Bass kernel for CachedMultiHeadAttention.

Problem: B=16, Q=32, KV=4096, D=1024, H=16 (DH=64), fp32 in/out.

Sharding: TP-4 x DP-2 hybrid. Core (dp, tp) handles batches [8dp, 8dp+8)
and heads [4tp, 4tp+4) = head-dim columns [256tp, 256tp+256).
Wq/Wk/Wv are column-split, Wo is row-split; each core emits a full-width
partial y for its 8 batches and the host sums the 4 TP partials per DP
group (and adds bo once). This cuts per-core weight DMA 16.8 MB -> 4.2 MB
while the mandatory 64 MiB/core KV stream stays untouched.

Per-core dataflow (per batch: 4 heads, 256 head-dims, 32 queries):
  - x^T via PE transpose; q materialized as per-batch block-diagonal fp16
    stationary operands (2 heads per [128, 64] tile) so one QK matmul per
    head-pair emits scores for 2 heads.
  - K stripes [512, 256] stream in f32, PE-transposed to fp16 K^T; QK in
    fp16. All PE transposes use a bf16 identity as the moving operand
    (1 cyc/row) with f32r-bitcast data.
  - Softmax skips max-subtraction (|scores*scale| small by construction).
  - exp(scores) transposed so W@V contracts over s on partitions; f32r
    W@V with a ones-column appended to V giving the denominator in col 256.
  - Per-batch: normalize, transpose to wv^T, project y^T = Wo_rows^T @ wv^T
    (no bias), transpose back, store 32 rows of the partial y. Streaming
    the output per batch keeps the post-DMA tail to a few microseconds.
"""

import numpy as np

import concourse.bass as bass
import concourse.bacc as bacc
import concourse.mybir as mybir
import concourse.tile as tile
from concourse.bass_utils import run_bass_kernel_spmd
from concourse.masks import make_identity

F32 = mybir.dt.float32
F32R = mybir.dt.float32r
BF16 = mybir.dt.bfloat16
FP16 = mybir.dt.float16

B, Q, KV, D, H = 16, 32, 4096, 1024, 16
DH = D // H                     # 64
NCORES = 8
NTP = 4                         # tensor-parallel ways (heads)
NDP = 2                         # data-parallel ways (batch)
NB = B // NDP                   # 8 batches per core
HPC = H // NTP                  # 4 heads per core
DSL = D // NTP                  # 256 head-dim slice per core
TOK = NB * Q                    # 256 tokens per core
SCALE = float(DH) ** -0.5       # folded q*k scale (DH**-0.25 applied twice)
NSTRIPE = 8                     # stripes of 512 cached s positions per batch
STRIPE = 512
GW = 260                        # v_aug stride (256 V + 2 ones + 2 pad)
NWV = 258                       # W@V moving size: 256 V cols + ones + dup


def _build_kernel():
    nc = bacc.Bacc(
        "TRN2",
        target_bir_lowering=False,
        debug=False,
        enable_asserts=False,
        num_devices=NCORES,
    )

    x_d = nc.dram_tensor("x", [TOK, D], F32R, kind="ExternalInput").ap()
    ck_d = nc.dram_tensor("cache_k", [NB, KV, DSL], F32R, kind="ExternalInput").ap()
    cv_d = nc.dram_tensor("cache_v", [NB, KV, DSL], F32R, kind="ExternalInput").ap()
    wq_d = nc.dram_tensor("Wq", [D, DSL], F32R, kind="ExternalInput").ap()
    wk_d = nc.dram_tensor("Wk", [D, DSL], F32R, kind="ExternalInput").ap()
    wv_d = nc.dram_tensor("Wv", [D, DSL], F32R, kind="ExternalInput").ap()
    wo_d = nc.dram_tensor("Wo", [DSL, D], F32R, kind="ExternalInput").ap()
    bq_d = nc.dram_tensor("bq", [DSL], F32, kind="ExternalInput").ap()
    bv_d = nc.dram_tensor("bv", [DSL], F32, kind="ExternalInput").ap()
    y_d = nc.dram_tensor("y", [TOK, D], F32, kind="ExternalOutput").ap()

    with tile.TileContext(nc) as tc:
        _body(tc, x_d, ck_d, cv_d, wq_d, wk_d, wv_d, wo_d, bq_d, bv_d, y_d)
    nc.compile()
    return nc


def _trT(nc, out_ap, in_ap, ident_r):
    """PE transpose in f32r (1.5 cyc/row vs 2.0 for plain f32). The input
    must come from a producer that wrote f32r (DMA / copy into an f32r
    tile) or the BIR verifier rejects the NEFF."""
    nc.tensor.matmul(
        out_ap.bitcast(F32R), in_ap.bitcast(F32R), ident_r,
        start=True, stop=True, is_transpose=True,
    )


def _trF(nc, out_ap, in_ap, ident):
    """Plain f32 PE transpose (2 cyc/row) for f32-produced data."""
    nc.tensor.matmul(
        out_ap, in_ap, ident, start=True, stop=True, is_transpose=True,
    )


def _body(tc, x_d, ck_d, cv_d, wq_d, wk_d, wv_d, wo_d, bq_d, bv_d, y_d):
    nc = tc.nc
    Exp = mybir.ActivationFunctionType.Exp

    with (
        tc.tile_pool(name="consts", bufs=1) as consts,
        tc.tile_pool(name="wo_pool", bufs=1) as wo_pool,
    ):
        identity = consts.tile([128, 128], F32)
        make_identity(nc, identity)
        ident_r = consts.tile([128, 128], F32R)
        nc.vector.tensor_copy(ident_r, identity)
        ones_row = consts.tile([1, TOK], F32)
        nc.vector.memset(ones_row, 1.0)

        bq_sb = consts.tile([1, DSL], F32)
        bv_sb = consts.tile([1, DSL], F32)

        x_sb = consts.tile([128, 2, D], F32R)
        # DMA issue order on the SP queue: x, Wq first (q projection gates
        # the first QK), then KV stripes stream from the main loop.
        nc.sync.dma_start(out=x_sb, in_=x_d.rearrange("(c p) d -> p c d", p=128))

        wo_sb = wo_pool.tile([128, 2, D], F32R)

        xT = consts.tile([128, 8, TOK], F32R)    # [d-part, d-chunk, tok]
        # block-diagonal fp16 q^T: chunk 2b+m holds batch b, head pair m:
        # rows 0:64 x cols 0:32 = even head, rows 64:128 x cols 32:64 = odd.
        qbd = consts.tile([128, 2 * NB, 64], FP16)
        kTc = consts.tile([128, 2, TOK], FP16)   # current-token K^T
        wvT = consts.tile([128, 2, TOK], F32R)   # attention output, transposed
        v_cur = consts.tile([Q, NB, GW], F32R)   # V_aug for current tokens

        # ---------------- stage A: x^T and projections ----------------
        with (
            tc.tile_pool(name="w3", bufs=1) as w3,
            tc.tile_pool(name="ppsum", bufs=3, space="PSUM") as ppsum,
        ):
            wq_sb = w3.tile([128, 8, DSL], F32R)
            nc.sync.dma_start(out=wq_sb, in_=wq_d.rearrange("(c p) m -> p c m", p=128))
            wk_sb = w3.tile([128, 8, DSL], F32R)
            wv_sb = w3.tile([128, 8, DSL], F32R)
            vT_sb = w3.tile([128, 2, TOK], F32R)
            nc.scalar.dma_start(out=bq_sb, in_=bq_d.rearrange("(a d) -> a d", a=1))
            nc.scalar.dma_start(out=bv_sb, in_=bv_d.rearrange("(a d) -> a d", a=1))
            nc.scalar.dma_start(out=wv_sb, in_=wv_d.rearrange("(c p) m -> p c m", p=128))
            nc.scalar.dma_start(out=wk_sb, in_=wk_d.rearrange("(c p) m -> p c m", p=128))
            nc.scalar.dma_start(out=wo_sb, in_=wo_d.rearrange("(c p) d -> p c d", p=128))

            # warmup op: first PE instruction depends only on the gpsimd
            # identity, so real work never accumulates a Pool wait.
            warm_ps = ppsum.tile([128, TOK], F32, tag="pp")
            nc.tensor.matmul(
                warm_ps[0:1, 0:1], identity[:, 0:1], identity[:, 0:1],
                start=True, stop=True,
            )
            for k in range(8):
                xt_ps = ppsum.tile([128, TOK], F32, tag="pp")
                for c in range(2):
                    _trT(nc, xt_ps[:, 128 * c : 128 * c + 128],
                         x_sb[:, c, 128 * k : 128 * k + 128], ident_r)
                nc.scalar.copy(out=xT[:, k, :], in_=xt_ps)

            nc.vector.memset(qbd, 0.0)
            for m in range(2):
                qp = ppsum.tile([128, TOK], F32, tag="pp")
                for k in range(8):
                    nc.tensor.matmul(
                        qp, wq_sb[:, k, 128 * m : 128 * m + 128], xT[:, k, :],
                        start=(k == 0), stop=False,
                    )
                nc.tensor.matmul(
                    qp, bq_sb[0:1, 128 * m : 128 * m + 128], ones_row,
                    start=False, stop=True,
                )
                for b in range(NB):
                    nc.scalar.copy(
                        out=qbd[0:64, 2 * b + m, 0:Q],
                        in_=qp[0:64, Q * b : Q * b + Q],
                    )
                    nc.scalar.copy(
                        out=qbd[64:128, 2 * b + m, Q : 2 * Q],
                        in_=qp[64:128, Q * b : Q * b + Q],
                    )

            for m in range(2):
                kp = ppsum.tile([128, TOK], F32, tag="pp")
                for k in range(8):
                    nc.tensor.matmul(
                        kp, wk_sb[:, k, 128 * m : 128 * m + 128], xT[:, k, :],
                        start=(k == 0), stop=(k == 7),
                    )
                nc.scalar.copy(out=kTc[:, m, :], in_=kp)

            nc.vector.memset(v_cur[:, :, 256:258].bitcast(F32), 1.0)
            for m in range(2):
                vp = ppsum.tile([128, TOK], F32, tag="pp")
                for k in range(8):
                    nc.tensor.matmul(
                        vp, wv_sb[:, k, 128 * m : 128 * m + 128], xT[:, k, :],
                        start=(k == 0), stop=False,
                    )
                nc.tensor.matmul(
                    vp, bv_sb[0:1, 128 * m : 128 * m + 128], ones_row,
                    start=False, stop=True,
                )
                nc.scalar.copy(out=vT_sb[:, m, :], in_=vp)
            for m in range(2):
                for b in range(NB):
                    vn_ps = ppsum.tile([128, TOK], F32, tag="ppn")
                    _trT(nc, vn_ps[0:Q, 0:128],
                         vT_sb[:, m, Q * b : Q * b + Q], ident_r)
                    nc.vector.tensor_copy(
                        v_cur[:, b, 128 * m : 128 * m + 128], vn_ps[0:Q, 0:128]
                    )

        # ---------------- main attention loop ----------------
        with (
            tc.tile_pool(name="knat", bufs=6) as knat_p,
            tc.tile_pool(name="ktp", bufs=2) as kt_p,
            tc.tile_pool(name="vaug", bufs=6) as vaug_p,
            tc.tile_pool(name="work", bufs=3) as work,
            tc.tile_pool(name="ybuf", bufs=2) as ybuf,
            tc.tile_pool(name="spsum", bufs=2, space="PSUM") as spsum,
            tc.tile_pool(name="trpsum", bufs=2, space="PSUM") as trpsum,
            tc.tile_pool(name="opsum", bufs=2, space="PSUM") as opsum,
            tc.tile_pool(name="ypsum", bufs=1, space="PSUM") as ypsum,
            tc.tile_pool(name="ytrp", bufs=1, space="PSUM") as ytrp,
        ):
            ck_r = [ck_d[b].rearrange("(j p) d -> p j d", p=128) for b in range(NB)]
            cv_r = [cv_d[b].rearrange("(j p) d -> p j d", p=128) for b in range(NB)]

            for b in range(NB):
                o_ps = opsum.tile([128, NWV], F32, tag="o_ps", name=f"o_b{b}")

                # current-token attention first: opens the accumulation
                # group without waiting on any cache stripe.
                scur_ps = spsum.tile([128, STRIPE], F32, tag="sc")
                for m in range(2):
                    nc.tensor.matmul(
                        scur_ps[64 * m : 64 * m + 64, 0:Q],
                        qbd[:, 2 * b + m, :],
                        kTc[:, m, Q * b : Q * b + Q],
                        start=True, stop=True,
                        tile_position=(0, 64 * m),
                    )
                w_cur = work.tile([128, Q], F32, tag="w_cur")
                nc.scalar.activation(w_cur, scur_ps[:, 0:Q], Exp, scale=SCALE)
                trc_ps = trpsum.tile([128, STRIPE], F32, tag="tr")
                _trF(nc, trc_ps[0:Q, 0:128], w_cur, identity)
                wt_cur = work.tile([Q, 128], F32R, tag="wt_cur")
                nc.vector.tensor_copy(wt_cur, trc_ps[0:Q, 0:128])
                nc.tensor.matmul(
                    o_ps, wt_cur, v_cur[:, b, 0:NWV],
                    start=True, stop=False, skip_group_check=True,
                )

                for S in range(NSTRIPE):
                    k_nat = knat_p.tile([128, 4, DSL], F32R)
                    nc.sync.dma_start(out=k_nat, in_=ck_r[b][:, 4 * S : 4 * S + 4, :])

                    v_aug = vaug_p.tile([128, 4, GW], F32R)
                    nc.vector.memset(v_aug[:, :, 256:258].bitcast(F32), 1.0)
                    # V stripes ride the idle Pool (SWDGE) queue so the SP
                    # sequencer only carries the K stream: one HWDGE setup
                    # (~1.2us) per 2.9us stripe instead of two.
                    nc.gpsimd.dma_start(
                        out=v_aug[:, :, 0:256],
                        in_=cv_r[b][:, 4 * S : 4 * S + 4, :],
                    )

                    kt = kt_p.tile([128, 2, STRIPE], FP16)
                    for dc in range(2):
                        tr_ps = trpsum.tile([128, STRIPE], F32, tag="tr")
                        for jj in range(4):
                            _trT(nc, tr_ps[:, 128 * jj : 128 * jj + 128],
                                 k_nat[:, jj, 128 * dc : 128 * dc + 128], ident_r)
                        if dc == 0:
                            nc.scalar.copy(out=kt[:, dc, :], in_=tr_ps)
                        else:
                            nc.vector.tensor_copy(kt[:, dc, :], tr_ps)

                    sc_ps = spsum.tile([128, STRIPE], F32, tag="sc")
                    for m in range(2):
                        nc.tensor.matmul(
                            sc_ps[64 * m : 64 * m + 64, :],
                            qbd[:, 2 * b + m, :],
                            kt[:, m, :],
                            start=True, stop=True,
                            tile_position=(0, 64 * m),
                        )
                    w_sb = work.tile([128, STRIPE], F32, tag="w_sb")
                    nc.scalar.activation(w_sb, sc_ps, Exp, scale=SCALE)

                    tr2_ps = trpsum.tile([128, STRIPE], F32, tag="tr")
                    for jj in range(4):
                        _trF(nc, tr2_ps[:, 128 * jj : 128 * jj + 128],
                             w_sb[:, 128 * jj : 128 * jj + 128], identity)
                    wt_sb = work.tile([128, STRIPE], F32R, tag="wt_sb")
                    nc.vector.tensor_copy(wt_sb, tr2_ps)

                    for jj in range(4):
                        nc.tensor.matmul(
                            o_ps,
                            wt_sb[:, 128 * jj : 128 * jj + 128],
                            v_aug[:, jj, 0:NWV],
                            start=False,
                            stop=(S == NSTRIPE - 1 and jj == 3),
                            skip_group_check=True,
                        )

                # normalize + extract into wv^T layout
                recip = work.tile([128, 1], F32, tag="recip")
                nc.vector.reciprocal(recip, o_ps[:, 256:257])
                o_sb = work.tile([128, 256], F32, tag="o_sb")
                nc.vector.tensor_scalar_mul(o_sb, o_ps[:, 0:256], recip)
                for u in range(2):
                    t_ps = trpsum.tile([128, STRIPE], F32, tag="tr")
                    _trF(nc, t_ps[:, 0:128], o_sb[:, 128 * u : 128 * u + 128],
                         identity)
                    nc.vector.tensor_copy(
                        wvT[0:64, u, Q * b : Q * b + Q],
                        t_ps[0:64, 64 * u : 64 * u + Q],
                    )
                    nc.vector.tensor_copy(
                        wvT[64:128, u, Q * b : Q * b + Q],
                        t_ps[64:128, 64 * u + Q : 64 * u + 2 * Q],
                    )

                # output projection for this batch: y^T = Wo_rows^T @ wv^T
                ytp = ypsum.tile([128, 256], F32, tag="yt")
                for mo in range(8):
                    for k in range(2):
                        nc.tensor.matmul(
                            ytp[:, 32 * mo : 32 * mo + 32],
                            wo_sb[:, k, 128 * mo : 128 * mo + 128],
                            wvT[:, k, Q * b : Q * b + Q],
                            start=(k == 0), stop=(k == 1),
                        )
                yT_b = work.tile([128, 256], F32R, tag="yT_b")
                nc.scalar.copy(out=yT_b, in_=ytp)
                y_b = ybuf.tile([Q, D], F32, tag="y_b")
                for g2 in range(2):
                    yn_ps = ytrp.tile([Q, STRIPE], F32, tag="yn")
                    for mo in range(4):
                        _trT(nc, yn_ps[0:Q, 128 * mo : 128 * mo + 128],
                             yT_b[:, 32 * (4 * g2 + mo) : 32 * (4 * g2 + mo) + 32],
                             ident_r)
                    nc.vector.tensor_copy(
                        y_b[:, 512 * g2 : 512 * g2 + 512], yn_ps
                    )
                nc.scalar.dma_start(out=y_d[Q * b : Q * b + Q, :], in_=y_b)


_NC_CACHE = None


def _get_nc():
    global _NC_CACHE
    if _NC_CACHE is None:
        _NC_CACHE = _build_kernel()
    return _NC_CACHE


def kernel(**inputs):
    x = np.asarray(inputs["x"], dtype=np.float32)
    ck = np.asarray(inputs["cache_k"], dtype=np.float32)
    cv = np.asarray(inputs["cache_v"], dtype=np.float32)
    Wq = np.asarray(inputs["Wq"], dtype=np.float32)
    Wk = np.asarray(inputs["Wk"], dtype=np.float32)
    Wv = np.asarray(inputs["Wv"], dtype=np.float32)
    Wo = np.asarray(inputs["Wo"], dtype=np.float32)
    bq = np.asarray(inputs["bq"], dtype=np.float32)
    bv = np.asarray(inputs["bv"], dtype=np.float32)
    bo = np.asarray(inputs["bo"], dtype=np.float32)

    nc = _get_nc()
    in_maps = []
    for c in range(NCORES):
        dp, tp = divmod(c, NTP)
        bs = slice(NB * dp, NB * dp + NB)
        ds = slice(DSL * tp, DSL * tp + DSL)
        in_maps.append({
            "x": np.ascontiguousarray(x[bs].reshape(TOK, D)),
            "cache_k": np.ascontiguousarray(ck[bs, :, ds]),
            "cache_v": np.ascontiguousarray(cv[bs, :, ds]),
            "Wq": np.ascontiguousarray(Wq[:, ds]),
            "Wk": np.ascontiguousarray(Wk[:, ds]),
            "Wv": np.ascontiguousarray(Wv[:, ds]),
            "Wo": np.ascontiguousarray(Wo[ds, :]),
            "bq": np.ascontiguousarray(bq[ds]),
            "bv": np.ascontiguousarray(bv[ds]),
        })

    res = run_bass_kernel_spmd(nc, in_maps, core_ids=list(range(NCORES)))
    global _LAST_RESULT
    _LAST_RESULT = res
    y = np.zeros((B, Q, D), dtype=np.float32)
    for c in range(NCORES):
        dp = c // NTP
        y[NB * dp : NB * dp + NB] += res.results[c]["y"].reshape(NB, Q, D)
    y += bo
    return y


_LAST_RESULT = None


# revision 9
# speedup vs baseline: 1.4147x; 1.4147x over previous
"""Trainium2 Bass kernel for CachedMultiHeadAttention.

Problem: B=16, Q=32, KV=4096, D=1024, H=16 (DH=64), fp32 in/out.

Sharding: TP-4 x DP-2 hybrid. Core (dp, tp) handles batches [8dp, 8dp+8)
and heads [4tp, 4tp+4) = head-dim columns [256tp, 256tp+256).
Wq/Wk/Wv are column-split, Wo is row-split; each core emits a full-width
partial y for its 8 batches and the host sums the 4 TP partials per DP
group (and adds bo once). This cuts per-core weight DMA 16.8 MB -> 4.2 MB
while the mandatory 64 MiB/core KV stream stays untouched.

Per-core dataflow (per batch: 4 heads, 256 head-dims, 32 queries):
  - x^T via PE transpose; q held as per-batch block-diagonal fp16 moving
    operands (2 heads per [128, 64] tile).
  - K stripes [512, 256] stream in f32, PE-transposed (f32r, 1.5 cyc/row)
    to fp16 K^T one stripe ahead of use.
  - Scores are computed TRANSPOSED: scT[s, hq] = kt_chunk^T @ qbd with
    K^T stationary. exp(scT) lands directly in W@V-stationary layout, so
    there is no per-stripe W transpose or extra PSUM->SBUF copy on the
    critical chain.
  - V converts f32->fp16 on the otherwise-idle GpSimd engine; W@V runs
    fp16 x fp16 with a ones-column giving the softmax denominator in
    column 256 of the accumulator.
  - Per-batch: normalize, transpose to wv^T, project y^T = Wo_rows^T @ wv^T
    (no bias), transpose back, store 32 rows of the partial y. Streaming
    the output per batch keeps the post-DMA tail to a few microseconds.
"""

import numpy as np

import concourse.bass as bass
import concourse.bacc as bacc
import concourse.mybir as mybir
import concourse.tile as tile
from concourse.bass_utils import run_bass_kernel_spmd
from concourse.masks import make_identity

F32 = mybir.dt.float32
F32R = mybir.dt.float32r
BF16 = mybir.dt.bfloat16
FP16 = mybir.dt.float16

B, Q, KV, D, H = 16, 32, 4096, 1024, 16
DH = D // H                     # 64
NCORES = 8
NTP = 4                         # tensor-parallel ways (heads)
NDP = 2                         # data-parallel ways (batch)
NB = B // NDP                   # 8 batches per core
HPC = H // NTP                  # 4 heads per core
DSL = D // NTP                  # 256 head-dim slice per core
TOK = NB * Q                    # 256 tokens per core
SCALE = float(DH) ** -0.5       # folded q*k scale (DH**-0.25 applied twice)
NSTRIPE = 8                     # stripes of 512 cached s positions per batch
STRIPE = 512
GW = 260                        # v_aug stride (256 V + 2 ones + 2 pad)
NWV = 258                       # W@V moving size: 256 V cols + ones + dup


def _build_kernel():
    nc = bacc.Bacc(
        "TRN2",
        target_bir_lowering=False,
        debug=False,
        enable_asserts=False,
        num_devices=NCORES,
    )

    x_d = nc.dram_tensor("x", [TOK, D], F32R, kind="ExternalInput").ap()
    ck_d = nc.dram_tensor("cache_k", [NB, KV, DSL], F32R, kind="ExternalInput").ap()
    cv_d = nc.dram_tensor("cache_v", [NB, KV, DSL], F32, kind="ExternalInput").ap()
    wq_d = nc.dram_tensor("Wq", [D, DSL], F32R, kind="ExternalInput").ap()
    wk_d = nc.dram_tensor("Wk", [D, DSL], F32R, kind="ExternalInput").ap()
    wv_d = nc.dram_tensor("Wv", [D, DSL], F32R, kind="ExternalInput").ap()
    wo_d = nc.dram_tensor("Wo", [DSL, D], F32R, kind="ExternalInput").ap()
    bq_d = nc.dram_tensor("bq", [DSL], F32, kind="ExternalInput").ap()
    bv_d = nc.dram_tensor("bv", [DSL], F32, kind="ExternalInput").ap()
    y_d = nc.dram_tensor("y", [TOK, D], F32, kind="ExternalOutput").ap()

    with tile.TileContext(nc) as tc:
        _body(tc, x_d, ck_d, cv_d, wq_d, wk_d, wv_d, wo_d, bq_d, bv_d, y_d)
    nc.compile()
    return nc


def _trT(nc, out_ap, in_ap, ident_r):
    """PE transpose in f32r (1.5 cyc/row vs 2.0 for plain f32). The input
    must come from a producer that wrote f32r (DMA / copy into an f32r
    tile) or the BIR verifier rejects the NEFF."""
    nc.tensor.matmul(
        out_ap.bitcast(F32R), in_ap.bitcast(F32R), ident_r,
        start=True, stop=True, is_transpose=True,
    )


def _trF(nc, out_ap, in_ap, ident):
    """Plain f32 PE transpose (2 cyc/row) for f32-produced data."""
    nc.tensor.matmul(
        out_ap, in_ap, ident, start=True, stop=True, is_transpose=True,
    )


def _body(tc, x_d, ck_d, cv_d, wq_d, wk_d, wv_d, wo_d, bq_d, bv_d, y_d):
    nc = tc.nc
    Exp = mybir.ActivationFunctionType.Exp

    with (
        tc.tile_pool(name="consts", bufs=1) as consts,
        tc.tile_pool(name="wo_pool", bufs=1) as wo_pool,
    ):
        identity = consts.tile([128, 128], F32)
        make_identity(nc, identity)
        ident_r = consts.tile([128, 128], F32R)
        nc.vector.tensor_copy(ident_r, identity)
        ones_row = consts.tile([1, TOK], F32)
        nc.vector.memset(ones_row, 1.0)

        bq_sb = consts.tile([1, DSL], F32)
        bv_sb = consts.tile([1, DSL], F32)

        x_sb = consts.tile([128, 2, D], F32R)
        # DMA issue order on the SP queue: x, Wq first (q projection gates
        # the first QK), then KV stripes stream from the main loop.
        nc.sync.dma_start(out=x_sb, in_=x_d.rearrange("(c p) d -> p c d", p=128))

        wo_sb = wo_pool.tile([128, 2, D], F32R)

        xT = consts.tile([128, 8, TOK], F32R)    # [d-part, d-chunk, tok]
        # block-diagonal fp16 q^T: chunk 2b+m holds batch b, head pair m:
        # rows 0:64 x cols 0:32 = even head, rows 64:128 x cols 32:64 = odd.
        qbd = consts.tile([128, 2 * NB, 64], FP16)
        kTc = consts.tile([128, 2, TOK], FP16)   # current-token K^T
        wvT = consts.tile([128, 2, TOK], F32R)   # attention output, transposed
        v_cur = consts.tile([Q, NB, GW], FP16)   # V_aug for current tokens

        # ---------------- stage A: x^T and projections ----------------
        with (
            tc.tile_pool(name="w3", bufs=1) as w3,
            tc.tile_pool(name="ppsum", bufs=3, space="PSUM") as ppsum,
        ):
            wq_sb = w3.tile([128, 8, DSL], F32R)
            nc.sync.dma_start(out=wq_sb, in_=wq_d.rearrange("(c p) m -> p c m", p=128))
            wk_sb = w3.tile([128, 8, DSL], F32R)
            wv_sb = w3.tile([128, 8, DSL], F32R)
            vT_sb = w3.tile([128, 2, TOK], F32R)
            nc.scalar.dma_start(out=bq_sb, in_=bq_d.rearrange("(a d) -> a d", a=1))
            nc.scalar.dma_start(out=bv_sb, in_=bv_d.rearrange("(a d) -> a d", a=1))
            nc.scalar.dma_start(out=wv_sb, in_=wv_d.rearrange("(c p) m -> p c m", p=128))
            nc.scalar.dma_start(out=wk_sb, in_=wk_d.rearrange("(c p) m -> p c m", p=128))
            nc.scalar.dma_start(out=wo_sb, in_=wo_d.rearrange("(c p) d -> p c d", p=128))

            # warmup op: first PE instruction depends only on the gpsimd
            # identity, so real work never accumulates a Pool wait.
            warm_ps = ppsum.tile([128, TOK], F32, tag="pp")
            nc.tensor.matmul(
                warm_ps[0:1, 0:1], identity[:, 0:1], identity[:, 0:1],
                start=True, stop=True,
            )
            for k in range(8):
                xt_ps = ppsum.tile([128, TOK], F32, tag="pp")
                for c in range(2):
                    _trT(nc, xt_ps[:, 128 * c : 128 * c + 128],
                         x_sb[:, c, 128 * k : 128 * k + 128], ident_r)
                nc.scalar.copy(out=xT[:, k, :], in_=xt_ps)

            nc.vector.memset(qbd, 0.0)
            for m in range(2):
                qp = ppsum.tile([128, TOK], F32, tag="pp")
                for k in range(8):
                    nc.tensor.matmul(
                        qp, wq_sb[:, k, 128 * m : 128 * m + 128], xT[:, k, :],
                        start=(k == 0), stop=False,
                    )
                nc.tensor.matmul(
                    qp, bq_sb[0:1, 128 * m : 128 * m + 128], ones_row,
                    start=False, stop=True,
                )
                for b in range(NB):
                    nc.scalar.copy(
                        out=qbd[0:64, 2 * b + m, 0:Q],
                        in_=qp[0:64, Q * b : Q * b + Q],
                    )
                    nc.scalar.copy(
                        out=qbd[64:128, 2 * b + m, Q : 2 * Q],
                        in_=qp[64:128, Q * b : Q * b + Q],
                    )

            for m in range(2):
                kp = ppsum.tile([128, TOK], F32, tag="pp")
                for k in range(8):
                    nc.tensor.matmul(
                        kp, wk_sb[:, k, 128 * m : 128 * m + 128], xT[:, k, :],
                        start=(k == 0), stop=(k == 7),
                    )
                nc.scalar.copy(out=kTc[:, m, :], in_=kp)

            nc.vector.memset(v_cur[:, :, 256:258], 1.0)
            for m in range(2):
                vp = ppsum.tile([128, TOK], F32, tag="pp")
                for k in range(8):
                    nc.tensor.matmul(
                        vp, wv_sb[:, k, 128 * m : 128 * m + 128], xT[:, k, :],
                        start=(k == 0), stop=False,
                    )
                nc.tensor.matmul(
                    vp, bv_sb[0:1, 128 * m : 128 * m + 128], ones_row,
                    start=False, stop=True,
                )
                nc.scalar.copy(out=vT_sb[:, m, :], in_=vp)
            for m in range(2):
                for b in range(NB):
                    vn_ps = ppsum.tile([128, TOK], F32, tag="ppn")
                    _trT(nc, vn_ps[0:Q, 0:128],
                         vT_sb[:, m, Q * b : Q * b + Q], ident_r)
                    nc.vector.tensor_copy(
                        v_cur[:, b, 128 * m : 128 * m + 128], vn_ps[0:Q, 0:128]
                    )

        # ---------------- main attention loop ----------------
        with (
            tc.tile_pool(name="knat", bufs=6) as knat_p,
            tc.tile_pool(name="vnat", bufs=6) as vnat_p,
            tc.tile_pool(name="ktp", bufs=2) as kt_p,
            tc.tile_pool(name="v16p", bufs=3) as v16_p,
            tc.tile_pool(name="work", bufs=3) as work,
            tc.tile_pool(name="ybuf", bufs=2) as ybuf,
            tc.tile_pool(name="spsum", bufs=2, space="PSUM") as spsum,
            tc.tile_pool(name="trpsum", bufs=2, space="PSUM") as trpsum,
            tc.tile_pool(name="opsum", bufs=2, space="PSUM") as opsum,
            tc.tile_pool(name="ypsum", bufs=1, space="PSUM") as ypsum,
            tc.tile_pool(name="ytrp", bufs=1, space="PSUM") as ytrp,
        ):
            ck_r = [ck_d[b].rearrange("(j p) d -> p j d", p=128) for b in range(NB)]
            cv_r = [cv_d[b].rearrange("(j p) d -> p j d", p=128) for b in range(NB)]
            NG = NB * NSTRIPE          # 64 global stripes

            def dma_kv(g):
                b, S = divmod(g, NSTRIPE)
                k_nat = knat_p.tile([128, 4, DSL], F32R, tag="k")
                nc.sync.dma_start(out=k_nat, in_=ck_r[b][:, 4 * S : 4 * S + 4, :])
                v_nat = vnat_p.tile([128, 4, DSL], F32, tag="v")
                nc.sync.dma_start(out=v_nat, in_=cv_r[b][:, 4 * S : 4 * S + 4, :])
                return k_nat, v_nat

            def prep_kt(k_nat):
                """PE-transpose a K stripe into fp16 K^T (one stripe ahead)."""
                kt = kt_p.tile([128, 2, STRIPE], FP16, tag="kt")
                for dc in range(2):
                    tr_ps = trpsum.tile([128, STRIPE], F32, tag="tr")
                    for jj in range(4):
                        _trT(nc, tr_ps[:, 128 * jj : 128 * jj + 128],
                             k_nat[:, jj, 128 * dc : 128 * dc + 128], ident_r)
                    if dc == 0:
                        nc.scalar.copy(out=kt[:, dc, :], in_=tr_ps)
                    else:
                        nc.vector.tensor_copy(kt[:, dc, :], tr_ps)
                return kt

            def prep_v16(v_nat):
                """f32 -> fp16 V conversion on the idle GpSimd engine."""
                v16 = v16_p.tile([128, 4, GW], FP16, tag="v16")
                nc.vector.memset(v16[:, :, 256:258], 1.0)
                nc.gpsimd.tensor_copy(v16[:, :, 0:256], v_nat)
                return v16

            # prologue: stripe 0 data + its kt/v16
            k_nat, v_nat = dma_kv(0)
            kt = prep_kt(k_nat)
            v16 = prep_v16(v_nat)

            o_ps = None
            for g in range(NG):
                b, S = divmod(g, NSTRIPE)
                if g + 1 < NG:
                    k_nat, v_nat = dma_kv(g + 1)

                if S == 0:
                    # current-token attention opens the accumulation group
                    o_ps = opsum.tile([128, NWV], F32, tag="o_ps")
                    scT_c = spsum.tile([128, STRIPE], F32, tag="sc")
                    for m in range(2):
                        nc.tensor.matmul(
                            scT_c[0:Q, 64 * m : 64 * m + 64],
                            kTc[:, m, Q * b : Q * b + Q],
                            qbd[:, 2 * b + m, :],
                            start=True, stop=True,
                        )
                    wT_c = work.tile([Q, 128], FP16, tag="wT_c")
                    nc.scalar.activation(wT_c, scT_c[0:Q, 0:128], Exp, scale=SCALE)
                    nc.tensor.matmul(
                        o_ps, wT_c, v_cur[:, b, 0:NWV],
                        start=True, stop=False, skip_group_check=True,
                    )

                # scores^T for stripe g via kt prepared last iteration
                scT = spsum.tile([128, STRIPE], F32, tag="sc")
                for jj in range(4):
                    for m in range(2):
                        nc.tensor.matmul(
                            scT[:, 128 * jj + 64 * m : 128 * jj + 64 * m + 64],
                            kt[:, m, 128 * jj : 128 * jj + 128],
                            qbd[:, 2 * b + m, :],
                            start=True, stop=True,
                        )
                wT = work.tile([128, STRIPE], FP16, tag="wT")
                nc.scalar.activation(wT, scT, Exp, scale=SCALE)

                # prepare next stripe's kt while exp runs
                if g + 1 < NG:
                    kt = prep_kt(k_nat)
                    nv16 = prep_v16(v_nat)

                for jj in range(4):
                    nc.tensor.matmul(
                        o_ps,
                        wT[:, 128 * jj : 128 * jj + 128],
                        v16[:, jj, 0:NWV],
                        start=False,
                        stop=(S == NSTRIPE - 1 and jj == 3),
                        skip_group_check=True,
                    )
                if g + 1 < NG:
                    v16 = nv16

                if S != NSTRIPE - 1:
                    continue

                # -------- batch finalize: normalize + wv^T + y projection ----
                recip = work.tile([128, 1], F32, tag="recip")
                nc.vector.reciprocal(recip, o_ps[:, 256:257])
                o_sb = work.tile([128, 256], F32, tag="o_sb")
                nc.vector.tensor_scalar_mul(o_sb, o_ps[:, 0:256], recip)
                for u in range(2):
                    t_ps = trpsum.tile([128, STRIPE], F32, tag="tr")
                    _trF(nc, t_ps[:, 0:128], o_sb[:, 128 * u : 128 * u + 128],
                         identity)
                    nc.vector.tensor_copy(
                        wvT[0:64, u, Q * b : Q * b + Q],
                        t_ps[0:64, 64 * u : 64 * u + Q],
                    )
                    nc.vector.tensor_copy(
                        wvT[64:128, u, Q * b : Q * b + Q],
                        t_ps[64:128, 64 * u + Q : 64 * u + 2 * Q],
                    )

                ytp = ypsum.tile([128, 256], F32, tag="yt")
                for mo in range(8):
                    for k in range(2):
                        nc.tensor.matmul(
                            ytp[:, 32 * mo : 32 * mo + 32],
                            wo_sb[:, k, 128 * mo : 128 * mo + 128],
                            wvT[:, k, Q * b : Q * b + Q],
                            start=(k == 0), stop=(k == 1),
                        )
                yT_b = work.tile([128, 256], F32R, tag="yT_b")
                nc.scalar.copy(out=yT_b, in_=ytp)
                y_b = ybuf.tile([Q, D], F32, tag="y_b")
                for g2 in range(2):
                    yn_ps = ytrp.tile([Q, STRIPE], F32, tag="yn")
                    for mo in range(4):
                        _trT(nc, yn_ps[0:Q, 128 * mo : 128 * mo + 128],
                             yT_b[:, 32 * (4 * g2 + mo) : 32 * (4 * g2 + mo) + 32],
                             ident_r)
                    nc.vector.tensor_copy(
                        y_b[:, 512 * g2 : 512 * g2 + 512], yn_ps
                    )
                nc.scalar.dma_start(out=y_d[Q * b : Q * b + Q, :], in_=y_b)


_NC_CACHE = None


def _get_nc():
    global _NC_CACHE
    if _NC_CACHE is None:
        _NC_CACHE = _build_kernel()
    return _NC_CACHE


def kernel(**inputs):
    x = np.asarray(inputs["x"], dtype=np.float32)
    ck = np.asarray(inputs["cache_k"], dtype=np.float32)
    cv = np.asarray(inputs["cache_v"], dtype=np.float32)
    Wq = np.asarray(inputs["Wq"], dtype=np.float32)
    Wk = np.asarray(inputs["Wk"], dtype=np.float32)
    Wv = np.asarray(inputs["Wv"], dtype=np.float32)
    Wo = np.asarray(inputs["Wo"], dtype=np.float32)
    bq = np.asarray(inputs["bq"], dtype=np.float32)
    bv = np.asarray(inputs["bv"], dtype=np.float32)
    bo = np.asarray(inputs["bo"], dtype=np.float32)

    nc = _get_nc()
    in_maps = []
    for c in range(NCORES):
        dp, tp = divmod(c, NTP)
        bs = slice(NB * dp, NB * dp + NB)
        ds = slice(DSL * tp, DSL * tp + DSL)
        in_maps.append({
            "x": np.ascontiguousarray(x[bs].reshape(TOK, D)),
            "cache_k": np.ascontiguousarray(ck[bs, :, ds]),
            "cache_v": np.ascontiguousarray(cv[bs, :, ds]),
            "Wq": np.ascontiguousarray(Wq[:, ds]),
            "Wk": np.ascontiguousarray(Wk[:, ds]),
            "Wv": np.ascontiguousarray(Wv[:, ds]),
            "Wo": np.ascontiguousarray(Wo[ds, :]),
            "bq": np.ascontiguousarray(bq[ds]),
            "bv": np.ascontiguousarray(bv[ds]),
        })

    res = run_bass_kernel_spmd(nc, in_maps, core_ids=list(range(NCORES)))
    global _LAST_RESULT
    _LAST_RESULT = res
    y = np.zeros((B, Q, D), dtype=np.float32)
    for c in range(NCORES):
        dp = c // NTP
        y[NB * dp : NB * dp + NB] += res.results[c]["y"].reshape(NB, Q, D)
    y += bo
    return y


_LAST_RESULT = None
